# revision 1
# baseline (speedup 1.0000x reference)
"""Trainium2 Bass kernel for nn_CustomNetwork_31585189494999 (gnn_message_passing).

Reference computation (per record b of B=65536, per non-root node n of 256):
  xg = x[:, parent_idx]                      # gather [B, 256, 8]
  h1 = einsum(xg, W1) + b1                   # per-node Linear(8->8)
  a1 = selu(batchnorm(h1))                   # BN over batch, per (node, ch)
  h2 = einsum(a1, W2) + b2                   # per-node Linear(8->8)
  a2 = selu(batchnorm(h2))
  out = einsum(a2, W3) + b3  ; sigmoid on last node only

Device strategy (8 NeuronCores, batch-sharded 8192 records/core):
  * Layer-1 matmul in "A-form": A1[c, (n,h)] = sum_d [parent=c] W1 -- a host-
    built (320 x 2048) scatter matrix; h1 = x @ A1 avoids any on-device gather.
    b1/b2 are dropped entirely (BatchNorm cancels additive per-channel biases).
  * BN1 statistics come from C = x^T x (one gram matmul + AllReduce) --
    E[h1^2] = diag(A1^T (C/B) A1), mean1 = mean_x @ A1. No pass over h1 needed.
  * SELU is computed branch-free:  selu(z)/LAM = relu(z) + AL*(e8(z) - 1),
    e8 = p(u)^32, u = clamp(min(z,0)/32, -0.5), p(u) = 1 + u + u^2/2.
    Two custom 8-stage DVE ops (POLY, COMBINE) evaluate this; relu(z) comes
    from the Scalar engine (Relu with per-partition BN scale/bias). The -AL
    constants cancel in the next BatchNorm / are host-folded into b3.
  * Layers 2/3 are block-diagonal matmuls (16 nodes x 8ch = 128 partitions).
  * h2 (raw) is spilled to HBM in fp16, stats (bn_stats/bn_aggr) AllReduced,
    then phase C reloads h2 and finishes BN2+SELU+Linear3.
  * Output is produced node-major [256, Bs] fp16 per core; host transposes.
"""

import math
import os
import sys

for _p in ("/opt/trn_rl_repo",):
    if _p not in sys.path:
        sys.path.insert(0, _p)

import numpy as np

import concourse.bass as bass
import concourse.mybir as mybir
import concourse.tile as tile
from concourse import bacc
from concourse.bass_utils import run_bass_kernel_spmd

F16 = mybir.dt.float16
F32 = mybir.dt.float32

NCORES = 8
NTOT = 320
NSUB = 256
DD = 8
HH = 8
CH = NSUB * HH            # 2048 channels
G = 16                    # channel groups of 128
XPAD = 384                # x feature pad (col 320 = ones, rest zero)
CHUNK = 1024              # batch tile width
EPS = 1e-5
LAM = 1.0507009873554805
AL = 1.6732632423543772
LN_AL = math.log(AL)

# engine-mix knobs: of every 16 (g,chunk) unit tiles, how many use the
# ScalarE exp path instead of the custom-DVE polynomial path.
MIX1 = int(os.environ.get("KMIX1", "0"))   # stage 1 (phase B)
MIX2 = int(os.environ.get("KMIX2", "0"))   # stage 2 (phase C)

_DVE_OPS = {}


def _ensure_dve_ops():
    """Register the two SELU custom DVE ops (idempotent, process-wide)."""
    if _DVE_OPS:
        return _DVE_OPS
    import concourse.dve_ops as dve_ops
    from concourse.dve_spec import (
        C0, C1, C2, C3, One, Spec, Src0, Src1, Zero,
        _spill_c3_to_src1, lower, minn, maxx,
    )
    from concourse.dve_spec import _has_src1 as has_src1
    from concourse.dve_uop import DveOpSpec

    def _pp(v, p):
        a = np.asarray(v, np.float32)
        if a.size == p:
            return a.reshape(p, 1)
        return np.float32(a.reshape(-1)[0]) if a.size else np.float32(0)

    # POLY: v = 1 + u + u^2/2,  u = max(min(Src0*C0 + C1, 0), C3)
    #   C0 = s/32 [P,1], C1 = t/32 [P,1], C2(imm) = 0.5, C3 -> in1 = clamp(-0.5)
    u = maxx(minn(Src0 * C0 + C1, Zero), C3)
    poly_body = _spill_c3_to_src1(One + u * (One + u * C2))

    def poly_ref(in0, in1, c0, c1, c2):
        x = in0.astype(np.float32)
        p = x.shape[0]
        sh = x.shape
        x2 = x.reshape(p, -1)
        c3v = _pp(in1, p)
        uu = np.maximum(np.minimum(x2 * _pp(c0, p) + _pp(c1, p), 0.0), c3v)
        return (1.0 + uu * (1.0 + uu * np.float32(c2))).astype(np.float32).reshape(sh)

    # COMBINE: y = Src0 + (Src1^32) * C2      (C2 imm = AL)
    def _sq(x):
        return x * x

    comb_body = Src0 + _sq(_sq(_sq(_sq(_sq(Src1))))) * C2

    def comb_ref(in0, in1, c0, c1, c2):
        r = in0.astype(np.float32)
        v = in1.astype(np.float32).reshape(r.shape[0], -1)
        return (r.reshape(v.shape) + (v ** 32) * np.float32(c2)).astype(
            np.float32).reshape(in0.shape)

    specs = [
        ("SELU_POLY_ANT", Spec(body=poly_body, reference=poly_ref)),
        ("SELU_COMB_ANT", Spec(body=comb_body, reference=comb_ref)),
    ]
    for name, spec in specs:
        if name not in dve_ops._SUB_OPCODE_FOR_NAME:
            row = dve_ops._CUSTOM_DVE_ROW_BASE + len(dve_ops.OPS)
            assert row < 0x20
            dve_ops._SUB_OPCODE_FOR_NAME[name] = row
            sha = {}
            for ver in ("v3",):
                s = DveOpSpec(name=name, opcode=row, uops=lower(spec, ver=ver),
                              rd1_en=has_src1(spec))
                sha[ver] = s.sha(ver)
            op = dve_ops.DveOp(name, spec, subdim=False, uops_sha=sha)
            dve_ops.OPS.append(op)
            dve_ops.CUSTOM_DVE_SPECS[name] = spec
        _DVE_OPS[name] = next(o for o in dve_ops.OPS if o.name == name)
    return _DVE_OPS


def _host_prep(inputs):
    """All precomputation that depends only on small inputs (and x packing)."""
    x = np.asarray(inputs["x"], np.float32)
    pidx = np.asarray(inputs["parent_idx"], np.int64)
    W1 = np.asarray(inputs["W1"], np.float32)
    W2 = np.asarray(inputs["W2"], np.float32)
    W3 = np.asarray(inputs["W3"], np.float32)
    b3 = np.asarray(inputs["b3"], np.float32)
    B = x.shape[0]

    # A-form scatter of W1: A1[c, n*8+h] = sum_d [pidx[n,d]==c] * W1[n,d,h]
    A1 = np.zeros((NTOT, CH), np.float32)
    for n in range(NSUB):
        for d in range(DD):
            A1[pidx[n, d], n * 8:(n + 1) * 8] += W1[n, d]

    # block-diagonal layer-2/3 weights (LAM folded in)
    w2blk = np.zeros((G, 128, 128), np.float32)
    w3blk = np.zeros((G, 128, 32), np.float32)
    for g in range(G):
        for j in range(16):
            n = 16 * g + j
            w2blk[g, 8 * j:8 * j + 8, 8 * j:8 * j + 8] = LAM * W2[n]
            w3blk[g, 8 * j:8 * j + 8, j] = LAM * W3[n, :, 0]

    # b3 with the two -LAM*AL constant folds:
    #   y2 we compute = LAM*(relu(z2) + AL*e8) = selu(z2) + LAM*AL
    #   -> out gets +LAM*AL * sum_h W3[n,h]; subtract it here.
    b3p = b3[:, 0] - LAM * AL * W3[:, :, 0].sum(axis=1)
    b3t = np.zeros((128, 4), np.float32)
    for t in range(4):
        for j in range(4):
            b3t[32 * j:32 * j + 16, t] = b3p[(4 * t + j) * 16:(4 * t + j) * 16 + 16]

    # per-group [128, 16] layouts of gamma/beta
    P = np.arange(128)
    gof = P // 8
    hof = P % 8

    def grouped(v):
        out = np.zeros((128, G), np.float32)
        for g in range(G):
            out[:, g] = v[16 * g + gof, hof]
        return out

    gb = np.stack([grouped(np.asarray(inputs[k], np.float32))
                   for k in ("gamma1", "beta1", "gamma2", "beta2")], axis=-1)

    # padded per-core x shards (col 320 = ones for the gram/mean trick)
    Bs = B // NCORES
    xpads = []
    for c in range(NCORES):
        xp = np.zeros((Bs, XPAD), np.float16)
        xp[:, :NTOT] = x[c * Bs:(c + 1) * Bs]
        xp[:, NTOT] = 1.0
        xpads.append(xp)

    common = {
        "a1w": A1.astype(np.float16),
        "w2blk": w2blk.astype(np.float16),
        "w3blk": w3blk.astype(np.float16),
        "b3t": b3t,
        "gb": gb,
    }
    return xpads, common, B, Bs


def build_body(tc, outs, ins, B, Bs):
    """Emit the whole program into TileContext `tc`.

    outs: {"out_T": AP [256, Bs] f16}
    ins:  {"xpad": [Bs, 384] f16, "a1w": [320, 2048] f16,
           "w2blk": [16,128,128] f16, "w3blk": [16,128,16] f16,
           "b3t": [128, 4] f32, "gb": [128, 16, 4] f32}
    """
    ops = _ensure_dve_ops()
    POLY, COMB = ops["SELU_POLY_ANT"], ops["SELU_COMB_ANT"]
    nc = tc.nc
    AF = mybir.ActivationFunctionType
    NCH = Bs // CHUNK
    KT = [(0, 128), (128, 128), (256, 64)]      # A1 / xT k-tiles
    CT = [(0, 128), (128, 128), (256, 65)]      # C-gram row tiles (incl. ones)
    xpad, a1w, w2blk, w3blk, b3t, gb = (ins[k] for k in
                                        ("xpad", "a1w", "w2blk", "w3blk", "b3t", "gb"))
    out_T = outs["out_T"]

    from contextlib import ExitStack
    stack = ExitStack()
    const = stack.enter_context(tc.tile_pool(name="const", bufs=1))
    dram = stack.enter_context(tc.tile_pool(name="dram", bufs=1, space="DRAM"))

    # ---- resident tensors -------------------------------------------------
    xt = [const.tile([128, Bs], F16, tag=f"xt{k}", name=f"xt{k}") for k in range(3)]
    for k in range(3):
        nc.sync.dma_start_transpose(xt[k][:], xpad[:, 128 * k:128 * (k + 1)])

    a1sb = [const.tile([sz, CH], F16, tag=f"a1_{k}", name=f"a1_{k}") for k, (of, sz) in enumerate(KT)]
    for k, (of, sz) in enumerate(KT):
        nc.sync.dma_start(a1sb[k][:], a1w[of:of + sz, :])

    w2sb = [const.tile([128, 128], F16, tag=f"w2_{g}", name=f"w2_{g}") for g in range(G)]
    w3sb = [const.tile([128, 32], F16, tag=f"w3_{g}", name=f"w3_{g}") for g in range(G)]
    for g in range(G):
        nc.sync.dma_start(w2sb[g][:], w2blk[g])
        nc.sync.dma_start(w3sb[g][:], w3blk[g])

    b3sb = const.tile([128, 4], F32, tag="b3")
    nc.sync.dma_start(b3sb[:], b3t[:])
    gbsb = const.tile([128, G, 4], F32, tag="gb")
    nc.sync.dma_start(gbsb[:], gb[:])

    onesb = const.tile([128, 1], F16, tag="ones")
    nc.vector.memset(onesb[:], 1.0)
    clampsb = const.tile([128, 1], F32, tag="clamp")
    nc.vector.memset(clampsb[:], -0.5)
    lnalsb = const.tile([128, 1], F32, tag="lnal")
    nc.vector.memset(lnalsb[:], LN_AL)

    # ---- phase 1: C = x^T x (gram, incl. ones column) ---------------------
    csb = [const.tile([sz, NTOT + 1], F16, tag=f"c_{m}", name=f"c_{m}") for m, (of, sz) in
           enumerate(CT)]
    with (tc.tile_pool(name="xbp", bufs=4) as xbp_pool,
          tc.tile_pool(name="cps", bufs=1, space="PSUM") as cps_pool):
        cps = [cps_pool.tile([sz, NTOT + 1], F32, tag=f"cps{m}", name=f"cps{m}")
               for m, (of, sz) in enumerate(CT)]
        nchb = Bs // 128
        for i in range(nchb):
            xb = xbp_pool.tile([128, XPAD], F16, tag="xb")
            nc.sync.dma_start(xb[:], xpad[128 * i:128 * (i + 1), :])
            for m, (of, sz) in enumerate(CT):
                nc.tensor.matmul(cps[m][:], xb[:, of:of + sz],
                                 xb[:, :NTOT + 1],
                                 start=(i == 0), stop=(i == nchb - 1))
        # local C / B -> f16, AllReduce via DRAM bounce
        cin = dram.tile([NTOT + 1, NTOT + 1], F16)
        cout = dram.tile([NTOT + 1, NTOT + 1], F16)
        for m, (of, sz) in enumerate(CT):
            cl = xbp_pool.tile([sz, NTOT + 1], F16, tag="cl")
            nc.scalar.activation(cl[:], cps[m][:], AF.Identity, scale=1.0 / B)
            nc.sync.dma_start(cin[of:of + sz, :], cl[:])
        nc.gpsimd.collective_compute(
            "AllReduce", mybir.AluOpType.add,
            replica_groups=[list(range(NCORES))],
            ins=[cin[:].opt()], outs=[cout[:].opt()])
        for m, (of, sz) in enumerate(CT):
            nc.sync.dma_start(csb[m][:], cout[of:of + sz, :])

    # ---- phase 3: BN1 parameters from C ----------------------------------
    # T = (C/B) @ A1 ; E[h1^2] = colsum(A1 * T[0:320]) ; mean1 = T[320]
    e2sb = const.tile([1, CH], F32, tag="e2row")
    meanrow = const.tile([1, CH], F16, tag="meanrow")
    with (tc.tile_pool(name="p3w", bufs=2) as p3w,
          tc.tile_pool(name="p3ps", bufs=1, space="PSUM") as p3ps):
        e2ps = p3ps.tile([1, CH], F32, tag="e2ps")
        # mean1 row: T[320, :] = sum_c C[320, c] * A1[c, :]
        mps = p3ps.tile([1, CH], F32, tag="tps")
        for nn in range(CH // 512):
            for k, (kof, ksz) in enumerate(KT):
                nc.tensor.matmul(
                    mps[:, 512 * nn:512 * (nn + 1)],
                    csb[k][:ksz, NTOT:NTOT + 1],
                    a1sb[k][:, 512 * nn:512 * (nn + 1)],
                    start=(k == 0), stop=(k == 2))
        nc.scalar.activation(meanrow[:], mps[:], AF.Copy)
        for m, (of, sz) in enumerate(KT):
            # T m-tile: rows of..of+sz of T = (C/B) @ A1
            tps = p3ps.tile([sz, CH], F32, tag="tps")
            for nn in range(CH // 512):
                for k, (kof, ksz) in enumerate(KT):
                    nc.tensor.matmul(tps[:, 512 * nn:512 * (nn + 1)],
                                     csb[k][:ksz, of:of + sz],
                                     a1sb[k][:, 512 * nn:512 * (nn + 1)],
                                     start=(k == 0), stop=(k == 2))
            tf = p3w.tile([sz, CH], F16, tag="tf")
            nc.scalar.activation(tf[:], tps[:], AF.Copy)
            prod = p3w.tile([sz, CH], F16, tag="prod")
            nc.vector.tensor_mul(prod[:], a1sb[m][:], tf[:])
            for nn in range(CH // 512):
                nc.tensor.matmul(e2ps[:, 512 * nn:512 * (nn + 1)],
                                 onesb[:sz, :],
                                 prod[:, 512 * nn:512 * (nn + 1)],
                                 start=(m == 0), stop=(m == 2))
        nc.vector.tensor_copy(e2sb[:], e2ps[:])

    # reshape [1, 2048] row -> [128, 16] group layout via a DRAM bounce
    # (SBUF free-dim data cannot be re-viewed across partitions directly)
    e2g = const.tile([128, G], F32, tag="e2g")
    m1g = const.tile([128, G], F32, tag="m1g")
    rowbounce = dram.tile([2, CH], F32)
    nc.gpsimd.dma_start(rowbounce[0:1, :], e2sb[:])
    nc.gpsimd.dma_start(rowbounce[1:2, :], meanrow[:])
    nc.sync.dma_start(e2g[:],
                      rowbounce[0:1, :].rearrange("o (g p) -> (o p) g", p=128))
    nc.sync.dma_start(m1g[:],
                      rowbounce[1:2, :].rearrange("o (g p) -> (o p) g", p=128))

    def bn_params(mean_t, e2_t, gamma_ap, beta_ap, pool):
        """-> (s, t, ns, nt, s32, t32) [128, G] f32 tiles."""
        var = pool.tile([128, G], F32, tag="var")
        nc.vector.tensor_mul(var[:], mean_t[:], mean_t[:])
        nc.vector.tensor_sub(var[:], e2_t[:], var[:])
        nc.vector.tensor_scalar_add(var[:], var[:], EPS)
        sq = pool.tile([128, G], F32, tag="sqv")
        nc.scalar.activation(sq[:], var[:], AF.Sqrt)
        r0 = pool.tile([128, G], F32, tag="r0")
        nc.vector.reciprocal(r0[:], sq[:])
        # one Newton step for rsqrt: r = r0*(1.5 - 0.5*var*r0^2)
        t1_ = pool.tile([128, G], F32, tag="nt1")
        nc.vector.tensor_mul(t1_[:], r0[:], r0[:])
        nc.vector.tensor_mul(t1_[:], var[:], t1_[:])
        nc.vector.tensor_scalar(t1_[:], t1_[:], -0.5, 1.5,
                                op0=mybir.AluOpType.mult, op1=mybir.AluOpType.add)
        rs = pool.tile([128, G], F32, tag="rs")
        nc.vector.tensor_mul(rs[:], r0[:], t1_[:])
        s = pool.tile([128, G], F32, tag="s")
        nc.vector.tensor_mul(s[:], gamma_ap, rs[:])
        t = pool.tile([128, G], F32, tag="t")
        nc.vector.tensor_mul(t[:], s[:], mean_t[:])
        nc.vector.tensor_sub(t[:], beta_ap, t[:])
        ns = pool.tile([128, G], F32, tag="ns")
        nc.vector.tensor_scalar_mul(ns[:], s[:], -1.0)
        nt = pool.tile([128, G], F32, tag="nt")
        nc.vector.tensor_scalar_mul(nt[:], t[:], -1.0)
        s32 = pool.tile([128, G], F32, tag="s32")
        nc.vector.tensor_scalar_mul(s32[:], s[:], 1.0 / 32.0)
        t32 = pool.tile([128, G], F32, tag="t32")
        nc.vector.tensor_scalar_mul(t32[:], t[:], 1.0 / 32.0)
        return s, t, ns, nt, s32, t32

    s1, t1, ns1, nt1, s1_32, t1_32 = bn_params(
        m1g, e2g, gbsb[:, :, 0].opt(), gbsb[:, :, 1].opt(), const)
    if "dbg" in outs:
        for i, tt_ in enumerate((e2g, m1g, s1, t1)):
            nc.gpsimd.dma_start(outs["dbg"][:, 16 * i:16 * (i + 1)], tt_[:])
        nc.gpsimd.dma_start(outs["dbgc"][:], csb[0][:])

    # ---- phase B: h1 -> selu -> h2 (spill + stats) ------------------------
    spill = dram.tile([G, 128, Bs], F16)
    bnstash = const.tile([128, G, 12 * NCH], F32, tag="bnstash")
    with (tc.tile_pool(name="qps", bufs=2, space="PSUM") as qps_pool,
          tc.tile_pool(name="hps", bufs=2, space="PSUM") as hps_pool,
          tc.tile_pool(name="wk", bufs=3) as wk):
        for c in range(NCH):
            for g in range(G):
                q = qps_pool.tile([128, CHUNK], F32, tag="q")
                for h in range(CHUNK // 512):
                    cof = c * CHUNK + 512 * h
                    for k, (of, sz) in enumerate(KT):
                        nc.tensor.matmul(
                            q[:, 512 * h:512 * (h + 1)],
                            a1sb[k][:, 128 * g:128 * (g + 1)],
                            xt[k][:sz, cof:cof + 512],
                            start=(k == 0), stop=(k == 2))
                r1 = wk.tile([128, CHUNK], F16, tag="r1")
                nc.scalar.activation(r1[:], q[:], AF.Relu,
                                     bias=t1[:, g:g + 1].opt(),
                                     scale=s1[:, g:g + 1].opt())
                y1 = wk.tile([128, CHUNK], F16, tag="y1")
                if (g + c) % 16 < MIX1:
                    mn = wk.tile([128, CHUNK], F16, tag="mn")
                    nc.scalar.activation(mn[:], q[:], AF.Relu,
                                         bias=nt1[:, g:g + 1].opt(),
                                         scale=ns1[:, g:g + 1].opt())
                    ex = wk.tile([128, CHUNK], F16, tag="ex")
                    nc.scalar.activation(ex[:], mn[:], AF.Exp,
                                         bias=lnalsb[:], scale=-1.0)
                    nc.vector.tensor_add(y1[:], r1[:], ex[:])
                else:
                    v = wk.tile([128, CHUNK], F32, tag="v")
                    nc.vector._custom_dve(POLY, out=v[:], in0=q[:],
                                          in1=clampsb[:],
                                          s0=s1_32[:, g:g + 1].opt(),
                                          s1=t1_32[:, g:g + 1].opt(), imm2=0.5)
                    nc.vector._custom_dve(COMB, out=y1[:], in0=r1[:],
                                          in1=v[:], imm2=AL)
                h2 = hps_pool.tile([128, CHUNK], F32, tag="h2")
                for h in range(CHUNK // 512):
                    nc.tensor.matmul(h2[:, 512 * h:512 * (h + 1)], w2sb[g][:],
                                     y1[:, 512 * h:512 * (h + 1)],
                                     start=True, stop=True)
                hq = wk.tile([128, CHUNK], F16, tag="hq")
                nc.scalar.activation(hq[:], h2[:], AF.Copy)
                for h in range(CHUNK // 512):
                    nc.vector.bn_stats(
                        bnstash[:, g, 12 * c + 6 * h:12 * c + 6 * (h + 1)].opt(),
                        h2[:, 512 * h:512 * (h + 1)])
                nc.sync.dma_start(spill[g, :, c * CHUNK:(c + 1) * CHUNK], hq[:])

    # ---- stats2 aggregation + AllReduce ----------------------------------
    mv = const.tile([128, G, 2], F32, tag="mv")
    for g in range(G):
        nc.vector.bn_aggr(mv[:, g, :].opt(), bnstash[:, g, :].opt())
    # convert to (sum, sumsq) * (Bs/B) so the AllReduce average is global
    ssq = const.tile([128, G, 2], F32, tag="ssq")
    nc.vector.tensor_mul(ssq[:, :, 0].opt(), mv[:, :, 0].opt(), mv[:, :, 0].opt())
    nc.vector.tensor_add(ssq[:, :, 1].opt(), mv[:, :, 1].opt(), ssq[:, :, 0].opt())
    nc.vector.tensor_scalar_mul(ssq[:, :, 1].opt(), ssq[:, :, 1].opt(), 1.0 / NCORES)
    nc.vector.tensor_scalar_mul(ssq[:, :, 0].opt(), mv[:, :, 0].opt(), 1.0 / NCORES)
    stin = dram.tile([128, G, 2], F32)
    stout = dram.tile([128, G, 2], F32)
    nc.sync.dma_start(stin[:], ssq[:])
    nc.gpsimd.collective_compute(
        "AllReduce", mybir.AluOpType.add, replica_groups=[list(range(NCORES))],
        ins=[stin[:].opt()], outs=[stout[:].opt()])
    gst = const.tile([128, G, 2], F32, tag="gst")
    nc.sync.dma_start(gst[:], stout[:])
    m2g = const.tile([128, G], F32, tag="m2g")
    e2g2 = const.tile([128, G], F32, tag="e2g2")
    nc.vector.tensor_copy(m2g[:], gst[:, :, 0].opt())
    nc.vector.tensor_copy(e2g2[:], gst[:, :, 1].opt())
    s2, t2, ns2, nt2, s2_32, t2_32 = bn_params(
        m2g, e2g2, gbsb[:, :, 2].opt(), gbsb[:, :, 3].opt(), const)
    if "dbg" in outs:
        for i, tt_ in enumerate((e2g2, m2g, s2, t2)):
            nc.gpsimd.dma_start(outs["dbg"][:, 64 + 16 * i:64 + 16 * (i + 1)], tt_[:])

    # ---- phase C: reload h2 -> selu -> out --------------------------------
    with (tc.tile_pool(name="ops2", bufs=2, space="PSUM") as ops_pool,
          tc.tile_pool(name="wk2", bufs=3) as wk2):
        for c in range(NCH):
            for t in range(4):
                op = ops_pool.tile([128, CHUNK], F32, tag="op")
                for j in range(4):
                    g = 4 * t + j
                    hq = wk2.tile([128, CHUNK], F16, tag="hq2")
                    nc.sync.dma_start(hq[:],
                                      spill[g, :, c * CHUNK:(c + 1) * CHUNK])
                    r2 = wk2.tile([128, CHUNK], F16, tag="r2")
                    nc.scalar.activation(r2[:], hq[:], AF.Relu,
                                         bias=t2[:, g:g + 1].opt(),
                                         scale=s2[:, g:g + 1].opt())
                    y2 = wk2.tile([128, CHUNK], F16, tag="y2")
                    if (g + c) % 16 < MIX2:
                        mn = wk2.tile([128, CHUNK], F16, tag="mn2")
                        nc.scalar.activation(mn[:], hq[:], AF.Relu,
                                             bias=nt2[:, g:g + 1].opt(),
                                             scale=ns2[:, g:g + 1].opt())
                        ex = wk2.tile([128, CHUNK], F16, tag="ex2")
                        nc.scalar.activation(ex[:], mn[:], AF.Exp,
                                             bias=lnalsb[:], scale=-1.0)
                        nc.vector.tensor_add(y2[:], r2[:], ex[:])
                    else:
                        v = wk2.tile([128, CHUNK], F32, tag="v2")
                        nc.vector._custom_dve(POLY, out=v[:], in0=hq[:],
                                              in1=clampsb[:],
                                              s0=s2_32[:, g:g + 1].opt(),
                                              s1=t2_32[:, g:g + 1].opt(),
                                              imm2=0.5)
                        nc.vector._custom_dve(COMB, out=y2[:], in0=r2[:],
                                              in1=v[:], imm2=AL)
                    for h in range(CHUNK // 512):
                        nc.tensor.matmul(
                            op[32 * j:32 * j + 32, 512 * h:512 * (h + 1)],
                            w3sb[g][:], y2[:, 512 * h:512 * (h + 1)],
                            start=True, stop=True, tile_position=(0, 32 * j))
                osb = wk2.tile([128, CHUNK], F16, tag="osb")
                nc.scalar.activation(osb[:], op[:], AF.Identity,
                                     bias=b3sb[:, t:t + 1].opt())
                for j in range(4):
                    if t == 3 and j == 3:
                        continue
                    nc.sync.dma_start(
                        out_T[64 * t + 16 * j:64 * t + 16 * (j + 1),
                              c * CHUNK:(c + 1) * CHUNK],
                        osb[32 * j:32 * j + 16, :])
                if t == 3:
                    # last 16-node block: node 255 needs a sigmoid; bounce its
                    # row to partition 0 (engines can't start at partition 111)
                    nc.sync.dma_start(
                        out_T[240:255, c * CHUNK:(c + 1) * CHUNK],
                        osb[96:111, :])
                    sgin = wk2.tile([1, CHUNK], F16, tag="sgin")
                    nc.sync.dma_start(sgin[:], osb[111:112, :])
                    sgout = wk2.tile([1, CHUNK], F16, tag="sgout")
                    nc.scalar.activation(sgout[:], sgin[:], AF.Sigmoid)
                    nc.sync.dma_start(
                        out_T[255:256, c * CHUNK:(c + 1) * CHUNK], sgout[:])
    stack.close()


def _bench(nc, in_maps, iters=6):
    """Time the NEFF execution via PJRT with pre-staged device inputs."""
    import time

    import jax
    from jax.experimental.shard_map import shard_map
    from jax.sharding import Mesh, PartitionSpec

    import concourse.mybir as mb
    from concourse import bass2jax as b2j

    b2j.install_neuronx_cc_hook()
    pname = nc.partition_id_tensor.name if nc.partition_id_tensor else None
    in_names, out_names, out_avals, zero_outs = [], [], [], []
    for alloc in nc.m.functions[0].allocations:
        if not isinstance(alloc, mb.MemoryLocationSet):
            continue
        name = alloc.memorylocations[0].name
        if alloc.kind == "ExternalInput":
            if name != pname:
                in_names.append(name)
        elif alloc.kind == "ExternalOutput":
            shape = tuple(alloc.tensor_shape)
            dtype = mb.dt.np(alloc.dtype)
            out_names.append(name)
            out_avals.append(jax.core.ShapedArray(shape, dtype))
            zero_outs.append(np.zeros(shape, dtype))
    n_params = len(in_names)
    all_names = in_names + out_names
    if pname is not None:
        all_names = all_names + [pname]

    def _body(*args):
        operands = list(args)
        if pname is not None:
            operands.append(b2j.partition_id_tensor())
        return tuple(b2j._bass_exec_p.bind(
            *operands, out_avals=tuple(out_avals), in_names=tuple(all_names),
            out_names=tuple(out_names), lowering_input_output_aliases=(),
            sim_require_finite=True, sim_require_nnan=True, nc=nc))

    devices = jax.devices()[:NCORES]
    mesh = Mesh(np.asarray(devices), ("core",))
    in_specs = (PartitionSpec("core"),) * (n_params + len(out_names))
    out_specs = (PartitionSpec("core"),) * len(out_names)
    fn = jax.jit(shard_map(_body, mesh=mesh, in_specs=in_specs,
                           out_specs=out_specs, check_rep=False))
    concat = [np.concatenate([np.asarray(m[nm]) for m in in_maps], axis=0)
              for nm in in_names]
    concat += [np.concatenate([z] * NCORES, axis=0) for z in zero_outs]
    sh = jax.sharding.NamedSharding(mesh, PartitionSpec("core"))
    dev_in = [jax.device_put(a, sh) for a in concat]
    outs = fn(*dev_in)
    jax.block_until_ready(outs)
    times = []
    for _ in range(iters):
        t0 = time.perf_counter()
        outs = fn(*dev_in)
        jax.block_until_ready(outs)
        times.append(time.perf_counter() - t0)
    return int(min(times) * 1e9)


_PROGRAM_CACHE = {}


def _build_program(B, Bs):
    key = (B, Bs, MIX1, MIX2, os.environ.get("KDBG", ""))
    if key in _PROGRAM_CACHE:
        return _PROGRAM_CACHE[key]
    nc = bacc.Bacc("TRN2", target_bir_lowering=False, debug=False,
                   enable_asserts=False, num_devices=NCORES)
    ins = {
        "xpad": nc.dram_tensor("xpad", [Bs, XPAD], F16, kind="ExternalInput").ap(),
        "a1w": nc.dram_tensor("a1w", [NTOT, CH], F16, kind="ExternalInput").ap(),
        "w2blk": nc.dram_tensor("w2blk", [G, 128, 128], F16,
                                kind="ExternalInput").ap(),
        "w3blk": nc.dram_tensor("w3blk", [G, 128, 32], F16,
                                kind="ExternalInput").ap(),
        "b3t": nc.dram_tensor("b3t", [128, 4], F32, kind="ExternalInput").ap(),
        "gb": nc.dram_tensor("gb", [128, G, 4], F32, kind="ExternalInput").ap(),
    }
    outs = {"out_T": nc.dram_tensor("out_T", [NSUB, Bs], F16,
                                    kind="ExternalOutput").ap()}
    if os.environ.get("KDBG"):
        outs["dbg"] = nc.dram_tensor("dbg", [128, 16 * 8], F32,
                                     kind="ExternalOutput").ap()
        outs["dbgc"] = nc.dram_tensor("dbgc", [128, NTOT + 1], F32,
                                      kind="ExternalOutput").ap()
    with tile.TileContext(nc) as tc:
        build_body(tc, outs, ins, B, Bs)
    nc.finalize()
    _PROGRAM_CACHE[key] = nc
    return nc


def kernel(**inputs) -> np.ndarray:
    xpads, common, B, Bs = _host_prep(inputs)
    nc = _build_program(B, Bs)
    in_maps = []
    for c in range(NCORES):
        m = {"xpad": xpads[c]}
        m.update(common)
        in_maps.append(m)
    res = run_bass_kernel_spmd(nc, in_maps, core_ids=list(range(NCORES)))
    kernel.last_results = res
    if os.environ.get("KBENCH"):
        kernel.bench_ns = _bench(nc, in_maps)
    out = np.concatenate(
        [np.asarray(r["out_T"]).T.astype(np.float32) for r in res.results], axis=0)
    return out

